# revision 1
# baseline (speedup 1.0000x reference)
"""ConvProduct forward (one-hot 2x2/stride-2 conv) as a Bass/Tile kernel on 8 trn2 cores.

Pure data parallel over batch (8 batches/core). Per batch:
  - one-hot weight W [128, 256] built on host from kernel_idx: partition
    p = kh*64 + a*32 + (kw*16 + cin)  (a = ho 32-block; kh blocks replicated per a)
  - DMA x[b] -> Q [128, 2048], partition p = kh*64 + ho, free (w, cin); 8KB/partition
    contiguous, single full-width DMA.
  - one DVE 32x32 block-transpose Q -> T: T[32A+i, 32B+j] with A = kh*2+a holds
    x[b, 2*(32a+j)+kh, (2B+kw)*16+cin], i = kw*16+cin. Each [32 x 128] slice of T is a
    K=32 lhsT for 128 output pixels (wo = 4c..4c+3, ho = 32a..32a+31) of kernel row kh.
  - TensorE one-hot matmuls perform the channel gather + transpose to pixel-major:
    psum[pix, o] accumulates kh=0 then kh=1 (K=32 row-tiled matmuls). The PE issue
    order is skewed one tile (mm0(c), mm0(c+1), mm1(c), ...) via explicit ordering
    deps so the two concurrent row-strips never write the same PSUM bank at once.
  - Evacuation PSUM->SBUF staging alternates ScalarE/VectorE; one full-width 2 MiB
    store DMA per (b, ho-half), 1KB contiguous per pixel.
"""
import os
import numpy as np

B, H, Wd, Cin = 64, 128, 128, 16
KH, KW, Cout = 2, 2, 256
Ho, Wo = 64, 64
NCORES = 8
BPC = B // NCORES

_CACHE = {}


def _build_nc(variant: str):
    import concourse.bass as bass
    import concourse.mybir as mybir
    import concourse.tile as tile
    from concourse import bacc
    from concourse.bass import _add_dep_helper as add_dep

    f32 = mybir.dt.float32
    nc = bacc.Bacc("TRN2", target_bir_lowering=False, debug=False)

    x = nc.dram_tensor("x", [BPC, H, Wd, Cin], f32, kind="ExternalInput")
    w = nc.dram_tensor("w", [128, Cout], f32, kind="ExternalInput")
    out = nc.dram_tensor("out", [BPC, Ho, Wo, Cout], f32, kind="ExternalOutput")

    with tile.TileContext(nc) as tc:
        with (
            tc.tile_pool(name="wp", bufs=1) as wp,
            tc.tile_pool(name="qp", bufs=3) as qp,
            tc.tile_pool(name="tp", bufs=2) as tp,
            tc.tile_pool(name="sp", bufs=3) as sp,
            tc.tile_pool(name="pp", bufs=6 if variant == "accum" else 4, space="PSUM") as pp,
        ):
            w_sb = wp.tile([128, Cout], f32)
            nc.sync.dma_start(w_sb[:], w.ap())

            prev_mm = [None]

            def mm(ps, A, c, **kw):
                inst = nc.tensor.matmul(
                    ps[:],
                    t[A * 32:(A + 1) * 32, c * 128:(c + 1) * 128],
                    w_sb[A * 32:(A + 1) * 32, :],
                    tile_position=(A * 32, 0),
                    skip_group_check=True,
                    **kw,
                )
                if prev_mm[0] is not None:
                    add_dep(inst.ins, prev_mm[0].ins, sync=False)
                prev_mm[0] = inst
                return inst

            for b in range(BPC):
                q = qp.tile([128, Wd * Cin], f32, tag="q")
                src = x.ap()[b].rearrange("(ho kh) w c -> kh ho (w c)", kh=2)
                nc.sync.dma_start(q[:], src)

                t = tp.tile([128, Wd * Cin], f32, tag="t")
                nc.vector.transpose(t[:], q[:])

                for a in range(2):
                    A0, A1 = a, 2 + a
                    st = sp.tile([128, 16 * Cout], f32, tag="st")

                    def evac(c, ps):
                        stsl = st[:, c * 256:(c + 1) * 256]
                        if c % 2 == 0:
                            nc.scalar.copy(stsl, ps[:])
                        else:
                            nc.vector.tensor_copy(stsl, ps[:])

                    if variant == "accum":
                        # 2-slot skew: strip A1's matmul on bank c starts only
                        # after strip A0 has moved 2 banks ahead, keeping it
                        # clear of mm0(c)'s ~128-cycle PSUM drain window.
                        SKEW = 2
                        ptiles = {}
                        for c in range(16):
                            ptiles[c] = pp.tile([128, 256], f32, tag="ps", name=f"ps_{b}_{a}_{c}")
                            mm(ptiles[c], A0, c, start=True, stop=False)
                            if c >= SKEW:
                                mm(ptiles[c - SKEW], A1, c - SKEW, start=False, stop=True)
                                evac(c - SKEW, ptiles.pop(c - SKEW))
                        for c in range(16 - SKEW, 16):
                            mm(ptiles[c], A1, c, start=False, stop=True)
                            evac(c, ptiles.pop(c))
                    else:  # "merge": kh partials in separate banks, add on evac
                        for cp in range(8):
                            c0, c1 = 2 * cp, 2 * cp + 1
                            p0 = pp.tile([128, 512], f32, tag="p0")
                            p1 = pp.tile([128, 512], f32, tag="p1")
                            for half, c in ((0, c0), (1, c1)):
                                nc.tensor.matmul(
                                    p0[:, half * 256:(half + 1) * 256],
                                    t[A0 * 32:(A0 + 1) * 32, c * 128:(c + 1) * 128],
                                    w_sb[A0 * 32:(A0 + 1) * 32, :],
                                    start=True, stop=True,
                                    tile_position=(A0 * 32, 0),
                                )
                            for half, c in ((0, c0), (1, c1)):
                                nc.tensor.matmul(
                                    p1[:, half * 256:(half + 1) * 256],
                                    t[A1 * 32:(A1 + 1) * 32, c * 128:(c + 1) * 128],
                                    w_sb[A1 * 32:(A1 + 1) * 32, :],
                                    start=True, stop=True,
                                    tile_position=(A1 * 32, 0),
                                )
                            stsl = st[:, c0 * 256:(c1 + 1) * 256]
                            nc.scalar.copy(stsl, p0[:])
                            nc.vector.tensor_add(stsl, p1[:], stsl)

                    dst = (
                        out.ap()[b]
                        .rearrange("(a hl) (c wl) o -> a wl hl c o", a=2, c=16)[a]
                    )
                    nc.sync.dma_start(dst, st[:])

    nc.compile()
    return nc


def _get_nc(variant: str | None = None):
    if variant is None:
        variant = os.environ.get("CONV_VARIANT", "accum")
    if variant not in _CACHE:
        _CACHE[variant] = _build_nc(variant)
    return _CACHE[variant]


def _build_w(kernel_idx: np.ndarray) -> np.ndarray:
    kidx = np.asarray(kernel_idx).astype(np.int64)
    w = np.zeros((128, Cout), np.float32)
    o = np.arange(Cout)
    for kh in range(KH):
        for a in range(2):
            for kw in range(KW):
                w[kh * 64 + a * 32 + kw * 16 + kidx[kh, kw], o] = 1.0
    return w


def kernel(x: np.ndarray, kernel_idx: np.ndarray) -> np.ndarray:
    from concourse.bass_utils import run_bass_kernel_spmd

    x = np.ascontiguousarray(np.asarray(x, dtype=np.float32))
    w = _build_w(kernel_idx)
    nc = _get_nc()

    in_maps = [
        {"x": x[c * BPC:(c + 1) * BPC], "w": w} for c in range(NCORES)
    ]
    res = run_bass_kernel_spmd(nc, in_maps, core_ids=list(range(NCORES)))
    kernel.last_results = res
    return np.concatenate([res.results[c]["out"] for c in range(NCORES)], axis=0)

